# revision 24
# baseline (speedup 1.0000x reference)
"""Trainium2 Bass kernel for nn_Decoder — v3 (see kernel.py docstring)."""
from contextlib import ExitStack

import numpy as np
import ml_dtypes

import concourse.bass as bass
import concourse.mybir as mybir
import concourse.tile as tile
from concourse import bacc
from concourse.bass_utils import run_bass_kernel_spmd
from concourse.masks import make_identity

F32 = mybir.dt.float32
F32R = mybir.dt.float32r
BF16 = mybir.dt.bfloat16
I32 = mybir.dt.int32
AF = mybir.ActivationFunctionType
ALU = mybir.AluOpType
AX = mybir.AxisListType

N_CORES = 8
B, T, U, E, VOCAB = 64, 128, 1024, 256, 32000
BL = B // N_CORES
R = BL * T
G2 = 2 * U
VS = VOCAB // N_CORES
NK = U // 128
NM = U // 128
NVN = 8
VN = VS // NVN
HB = 4

CFG = {
    "mm1": "bf16",
    "scores": "bf16",
    "gru": "bf16",
    "fc": "bf16",
}


def _dt(stage):
    return BF16 if CFG[stage] == "bf16" else F32R


def _np_dt(stage):
    return ml_dtypes.bfloat16 if CFG[stage] == "bf16" else np.float32


def build_nc():
    nc = bacc.Bacc("TRN2", target_bir_lowering=False, debug=False,
                   num_devices=N_CORES)

    d_mm1, d_sc, d_gru, d_fc = (_dt(s) for s in ("mm1", "scores", "gru", "fc"))

    enc_t = nc.dram_tensor("enc_t", [U, R], d_mm1, kind="ExternalInput").ap()
    w1 = nc.dram_tensor("w1", [U, U], d_mm1, kind="ExternalInput").ap()
    w2 = nc.dram_tensor("w2", [U, U], d_mm1, kind="ExternalInput").ap()
    hidden_t = nc.dram_tensor("hidden_t", [U, BL], d_mm1, kind="ExternalInput").ap()
    b1n = nc.dram_tensor("b1n", [1, U], F32, kind="ExternalInput").ap()
    b2n = nc.dram_tensor("b2n", [1, U], F32, kind="ExternalInput").ap()
    sel = nc.dram_tensor("sel", [2, BL + 1, 512], F32R, kind="ExternalInput").ap()
    v_t = nc.dram_tensor("v_t", [128, NM], d_sc, kind="ExternalInput").ap()
    emb = nc.dram_tensor("emb", [VOCAB, E], F32, kind="ExternalInput").ap()
    x_idx = nc.dram_tensor("x_idx", [B, 1], I32, kind="ExternalInput").ap()
    gru_k = nc.dram_tensor("gru_k_zh", [NK + 2, 128, G2], d_gru,
                           kind="ExternalInput").ap()
    gru_b = nc.dram_tensor("gru_b_zh", [1, G2], F32, kind="ExternalInput").ap()
    fc_w = nc.dram_tensor("fc_w", [U, VS], d_fc, kind="ExternalInput").ap()
    fc_b = nc.dram_tensor("fc_b", [1, VS], d_fc, kind="ExternalInput").ap()

    out_logits = nc.dram_tensor("out_logits", [B, VS], F32, kind="ExternalOutput").ap()
    out_state = nc.dram_tensor("out_state", [B, U], F32, kind="ExternalOutput").ap()
    out_attn = nc.dram_tensor("out_attn", [BL, T], F32, kind="ExternalOutput").ap()

    def bcast(src_ap, parts):
        inner = [list(d) for d in src_ap.ap if d[1] != 1]
        return bass.AP(tensor=src_ap.tensor, offset=src_ap.offset,
                       ap=[[0, parts]] + inner)

    with tile.TileContext(nc) as tc, ExitStack() as es:
        consts = es.enter_context(tc.tile_pool(name="consts", bufs=1))
        enc_p = es.enter_context(tc.tile_pool(name="enc", bufs=1))
        w1_p = es.enter_context(tc.tile_pool(name="w1", bufs=1))
        w2_p = es.enter_context(tc.tile_pool(name="w2", bufs=2))
        tanh_p = es.enter_context(tc.tile_pool(name="tanh", bufs=1))
        fcw_p = es.enter_context(tc.tile_pool(name="fcw", bufs=4))
        gruk_p = es.enter_context(tc.tile_pool(name="gruk", bufs=5))
        small = es.enter_context(tc.tile_pool(name="small", bufs=1))
        psum = es.enter_context(tc.tile_pool(name="psum", bufs=8, space="PSUM"))
        dram = es.enter_context(tc.tile_pool(name="dram", bufs=4, space="DRAM"))

        # ---- phase 0: loads (few, large), h2, embedding ----
        hid_sb = consts.tile([128, NK, BL], d_mm1)
        nc.sync.dma_start(out=hid_sb[:],
                          in_=hidden_t.rearrange("(k p) b -> p k b", p=128))
        w2_sb = []
        for j in range(4):
            t = w2_p.tile([128, 2, U], d_mm1, tag="w2", name=f"w2_{j}")
            nc.sync.dma_start(
                out=t[:],
                in_=w2[j * 256:(j + 1) * 256, :].rearrange(
                    "(k p) n -> p k n", p=128))
            w2_sb.append(t)
        enc_sb = enc_p.tile([128, NK, R], d_mm1, tag="enc")
        for j in range(2):
            nc.sync.dma_start(
                out=enc_sb[:, j * 4:(j + 1) * 4, :],
                in_=enc_t[j * 512:(j + 1) * 512, :].rearrange(
                    "(k p) r -> p k r", p=128))
        w1_sb = consts.tile([128, NK, U], d_mm1)
        for j in range(2):
            nc.sync.dma_start(
                out=w1_sb[:, j * 4:(j + 1) * 4, :],
                in_=w1[j * 512:(j + 1) * 512, :].rearrange(
                    "(k p) n -> p k n", p=128))

        ident = consts.tile([128, 128], F32)
        make_identity(nc, ident[:])
        ones = consts.tile([1, B], d_fc)
        nc.vector.memset(ones[:], 1.0)
        v_sb = consts.tile([128, NM], d_sc)
        nc.sync.dma_start(out=v_sb[:], in_=v_t[:])
        sel_sb = consts.tile([BL + 1, 2, 512], F32R)
        nc.sync.dma_start(out=sel_sb[:],
                          in_=sel.rearrange("h j n -> j h n"))
        b1_sb = consts.tile([1, U], F32)
        nc.sync.dma_start(out=b1_sb[:], in_=b1n[:])
        b2_sb = consts.tile([1, U], F32)
        nc.sync.dma_start(out=b2_sb[:], in_=b2n[:])

        # h2 = hidden @ W2 (natural [BL, U]); pack into f32r-rounded h2aug:
        # rows 0-3 = h2[b0..3], row 4 = b1+b2, rows 5-8 = h2[b4..7], row 9 = b1+b2
        h2aug = consts.tile([BL + 1, U], F32R)
        ph = [psum.tile([BL, 512], F32, tag="pb", name=f"ph{n}") for n in range(2)]
        for k in range(NK):
            for n in range(2):
                nc.tensor.matmul(
                    ph[n][:],
                    hid_sb[:, k, :],
                    w2_sb[k // 2][:, k % 2, n * 512:(n + 1) * 512],
                    start=(k == 0), stop=(k == NK - 1))
        for n in range(2):
            nc.vector.tensor_copy(out=h2aug[0:BL, n * 512:(n + 1) * 512],
                                  in_=ph[n][:])
        b12 = small.tile([1, U], F32, tag="b12")
        nc.vector.tensor_add(out=b12[:], in0=b1_sb[:], in1=b2_sb[:])
        nc.sync.dma_start(out=h2aug[BL:BL + 1, :], in_=b12[:].bitcast(F32R))

        # full-batch embedding gather -> gru input chunks 8..9 [128, B]
        idx_sb = small.tile([B, 1], I32, tag="idx")
        nc.sync.dma_start(out=idx_sb[:], in_=x_idx[:])
        xe_sb = small.tile([B, E], F32, tag="xe")
        nc.gpsimd.indirect_dma_start(
            out=xe_sb[:], out_offset=None, in_=emb[:],
            in_offset=bass.IndirectOffsetOnAxis(ap=idx_sb[:, :1], axis=0))
        gx = []
        for j in range(2):
            pt = psum.tile([128, B], F32, tag="pb", name=f"xe_ps{j}")
            nc.tensor.transpose(pt[:], xe_sb[:, j * 128:(j + 1) * 128],
                                ident[:B, :B])
            g = small.tile([128, B], d_gru, tag=f"gx{j}", name=f"gx{j}")
            nc.vector.tensor_copy(out=g[:], in_=pt[:])
            gx.append(g)

        # ---- phase 1: attention, two 512-row halves ----
        tanh_sb = tanh_p.tile([128, NM, R], d_sc, tag="tanh")
        sc_dram = dram.tile([1, R], F32)
        attn_dram = dram.tile([BL, T], F32)
        st_gin = small.tile([128, NK, BL], F32, tag="stgin")
        for h in range(2):
            rows = slice(h * 512, (h + 1) * 512)
            for m in range(NM):
                ps = psum.tile([128, 512], F32, tag="pb", name=f"mm1_{h}_{m}")
                for k in range(NK):
                    nc.tensor.matmul(
                        ps[:],
                        w1_sb[:, k, m * 128:(m + 1) * 128],
                        enc_sb[:, k, rows],
                        start=(k == 0), stop=False)
                nc.tensor.matmul(
                    ps[:],
                    h2aug[:, m * 128:(m + 1) * 128],
                    sel_sb[:, h, :],
                    start=False, stop=True)
                nc.scalar.activation(out=tanh_sb[:, m, rows], in_=ps[:],
                                     func=AF.Tanh)
            psc = psum.tile([1, 512], F32, tag="pb", name=f"sc_ps{h}")
            for m in range(NM):
                nc.tensor.matmul(
                    psc[:], v_sb[:, m:m + 1], tanh_sb[:, m, rows],
                    start=(m == 0), stop=(m == NM - 1))
            sc_sb = small.tile([1, 512], F32, tag="scsb", bufs=2,
                               name=f"sc_sb{h}")
            nc.vector.tensor_copy(out=sc_sb[:], in_=psc[:])
            nc.sync.dma_start(out=sc_dram[:, rows], in_=sc_sb[:])

            sm_sb = small.tile([HB, T], F32, tag="smx", bufs=2, name=f"sm{h}")
            nc.sync.dma_start(
                out=sm_sb[:],
                in_=sc_dram[0:1, rows].rearrange("o (b t) -> (o b) t", t=T))
            # softmax without max-subtraction (scores are O(1); exp is safe)
            ex_sb = small.tile([HB, T], F32, tag="ex", bufs=2, name=f"ex{h}")
            esum = small.tile([HB, 1], F32, tag="esum", bufs=2, name=f"esum{h}")
            nc.scalar.activation(out=ex_sb[:], in_=sm_sb[:], func=AF.Exp,
                                 accum_out=esum[:, :1])
            rsum = small.tile([HB, 1], F32, tag="rsum", bufs=2, name=f"rsum{h}")
            nc.vector.reciprocal(out=rsum[:], in_=esum[:])
            attn_sb = small.tile([HB, T], F32, tag="attn", bufs=2,
                                 name=f"attn{h}")
            nc.vector.tensor_scalar_mul(out=attn_sb[:], in0=ex_sb[:],
                                        scalar1=rsum[:, :1])
            nc.sync.dma_start(out=out_attn[h * HB:(h + 1) * HB, :], in_=attn_sb[:])
            nc.sync.dma_start(out=attn_dram[h * HB:(h + 1) * HB, :], in_=attn_sb[:])

            abc = small.tile([128, HB, T], F32, tag="abc", bufs=2,
                             name=f"abc{h}")
            nc.gpsimd.dma_start(out=abc[:],
                                in_=bcast(attn_dram[h * HB:(h + 1) * HB, :], 128))
            scratch = small.tile([128, T], F32, tag="scr", bufs=2,
                                 name=f"scr{h}")
            for b in range(HB):
                gb = h * HB + b
                for k in range(NK):
                    nc.vector.scalar_tensor_tensor(
                        out=scratch[:],
                        in0=enc_sb[:, k, gb * T:(gb + 1) * T],
                        scalar=1.0, in1=abc[:, b, :],
                        op0=ALU.mult, op1=ALU.mult,
                        accum_out=st_gin[:, k, gb:gb + 1])

        gb_bc = small.tile([B, G2], F32, tag="gbbc")
        nc.gpsimd.dma_start(out=gb_bc[:], in_=bcast(gru_b[0:1, :], B))

        # ---- phase 2: all-gather of contextT ----
        st_bf = small.tile([128, NK, BL], d_gru, tag="stbf")
        nc.vector.tensor_copy(out=st_bf[:], in_=st_gin[:])
        cc_in = dram.tile([U, BL], d_gru)
        nc.sync.dma_start(out=cc_in[:].rearrange("(k p) b -> p k b", p=128),
                          in_=st_bf[:])
        cc_out = dram.tile([N_CORES, U, BL], d_gru)
        nc.gpsimd.collective_compute(
            "AllGather", ALU.bypass,
            replica_groups=[list(range(N_CORES))],
            ins=[cc_in.opt()], outs=[cc_out.opt()])

        # stream gru_k / fc_w while the collective runs (issued after the
        # collective so their transfers fill its window; sync queue — the
        # scalar HWDGE path is broken on this stack)
        gruk_sb = []
        for k in range(NK + 2):
            gt = gruk_p.tile([128, G2], d_gru, tag="gruk", name=f"gruk{k}")
            nc.sync.dma_start(out=gt[:], in_=gru_k[k])
            gruk_sb.append(gt)
        fcw_sb = []
        for k in range(NK):
            t = fcw_p.tile([128, VS], d_fc, tag="fcw", name=f"fcw{k}")
            nc.sync.dma_start(out=t[:], in_=fc_w[k * 128:(k + 1) * 128, :])
            fcw_sb.append(t)
        fcb_sb = small.tile([1, VS], d_fc, tag="fcb")
        nc.sync.dma_start(out=fcb_sb[:], in_=fc_b[:])
        gf_all = small.tile([128, NK, B], d_gru, tag="gfall")
        for k in range(NK):
            nc.sync.dma_start(
                out=gf_all[:, k, :].rearrange("p (r b) -> p r b", r=N_CORES),
                in_=cc_out[:, k * 128:(k + 1) * 128, :].rearrange(
                    "r p b -> p r b"))

        # ---- phase 3: full-batch GRU (replicated on every core) ----
        gin_ap = [gf_all[:, k, :] for k in range(NK)] + [gx[0][:], gx[1][:]]
        pg = [psum.tile([B, 512], F32, tag="pb", name=f"pg{n}")
              for n in range(4)]
        for k in range(NK + 2):
            for n in range(4):
                nc.tensor.matmul(
                    pg[n][:], gin_ap[k],
                    gruk_sb[k][:, n * 512:(n + 1) * 512],
                    start=(k == 0), stop=(k == NK + 1))
        z_sb = small.tile([B, U], F32, tag="z")
        hh_sb = small.tile([B, U], F32, tag="hh")
        for n in range(4):
            gsum = small.tile([B, 512], F32, tag="gsum", bufs=2,
                              name=f"gsum{n}")
            nc.vector.tensor_add(out=gsum[:], in0=pg[n][:],
                                 in1=gb_bc[:, n * 512:(n + 1) * 512])
            if n < 2:
                nc.scalar.activation(out=z_sb[:, n * 512:(n + 1) * 512],
                                     in_=gsum[:], func=AF.Sigmoid)
            else:
                nc.scalar.activation(out=hh_sb[:, (n - 2) * 512:(n - 1) * 512],
                                     in_=gsum[:], func=AF.Tanh)
        state_sb = small.tile([B, U], F32, tag="state")
        nc.vector.tensor_mul(out=z_sb[:], in0=z_sb[:], in1=hh_sb[:])
        nc.vector.tensor_sub(out=state_sb[:], in0=hh_sb[:], in1=z_sb[:])
        nc.sync.dma_start(out=out_state[:], in_=state_sb[:])

        stT = []
        for m in range(NM):
            pt = psum.tile([128, B], F32, tag="pb", name=f"st_ps{m}")
            nc.tensor.transpose(pt[:], state_sb[:, m * 128:(m + 1) * 128],
                                ident[:B, :B])
            sb_ = small.tile([128, B], d_fc, tag="sTb", bufs=NM,
                             name=f"stT{m}")
            nc.vector.tensor_copy(out=sb_[:], in_=pt[:])
            stT.append(sb_)

        # ---- phase 4: fc projection ----
        lo_all = small.tile([B, VS], F32, tag="loall")
        pl = [psum.tile([B, VN], F32, tag="pb", name=f"pl{n}")
              for n in range(NVN)]
        for k in range(NK):
            for n in range(NVN):
                nc.tensor.matmul(
                    pl[n][:], stT[k][:],
                    fcw_sb[k][:, n * VN:(n + 1) * VN],
                    start=(k == 0), stop=False)
        for n in range(NVN):
            nc.tensor.matmul(
                pl[n][:], ones[:], fcb_sb[:, n * VN:(n + 1) * VN],
                start=False, stop=True)
            nc.vector.tensor_copy(out=lo_all[:, n * VN:(n + 1) * VN],
                                  in_=pl[n][:])
        nc.sync.dma_start(out=out_logits[:], in_=lo_all[:])

    nc.compile()
    return nc


def shard_inputs(x, hidden, enc_output, emb, W1, b1, W2, b2, V, bV,
                 gru_k, gru_rk, gru_b, fc_W, fc_b):
    f32 = np.float32
    d_mm1, d_sc, d_gru, d_fc = (_np_dt(s) for s in ("mm1", "scores", "gru", "fc"))

    x = np.asarray(x).astype(np.int32).reshape(B, 1)
    hidden = np.asarray(hidden, f32)
    enc_output = np.asarray(enc_output, f32)
    emb_np = np.ascontiguousarray(np.asarray(emb, f32))
    W1_np = np.ascontiguousarray(np.asarray(W1, f32).astype(d_mm1))
    W2_np = np.ascontiguousarray(np.asarray(W2, f32).astype(d_mm1))
    b1n = np.ascontiguousarray(np.asarray(b1, f32)[None, :])
    b2n = np.ascontiguousarray(np.asarray(b2, f32)[None, :])
    selv = np.zeros((2, BL + 1, 512), f32)
    for h in range(2):
        for j in range(HB):
            selv[h, h * HB + j, j * T:(j + 1) * T] = 1.0
        selv[h, BL, :] = 1.0
    v_t = np.ascontiguousarray(np.asarray(V, f32)[:, 0].reshape(NM, 128).T
                               .astype(d_sc))
    gk = np.asarray(gru_k, f32)
    gru_k_zh = np.ascontiguousarray(
        np.concatenate([gk[:, :U], gk[:, 2 * U:]], axis=1).astype(d_gru)
        .reshape(NK + 2, 128, G2))
    gb = np.asarray(gru_b, f32)
    gru_b_zh = np.ascontiguousarray(
        np.concatenate([gb[:U], gb[2 * U:]])[None, :].astype(f32))
    fc_W_np = np.asarray(fc_W, f32)
    fc_b_np = np.asarray(fc_b, f32)

    in_maps = []
    for c in range(N_CORES):
        sl = slice(c * BL, (c + 1) * BL)
        enc_c = enc_output[sl].reshape(R, U)
        in_maps.append({
            "enc_t": np.ascontiguousarray(enc_c.T).astype(d_mm1),
            "w1": W1_np,
            "w2": W2_np,
            "hidden_t": np.ascontiguousarray(hidden[sl].T).astype(d_mm1),
            "b1n": b1n,
            "b2n": b2n,
            "sel": selv,
            "v_t": v_t,
            "emb": emb_np,
            "x_idx": x,
            "gru_k_zh": gru_k_zh,
            "gru_b_zh": gru_b_zh,
            "fc_w": np.ascontiguousarray(
                fc_W_np[:, c * VS:(c + 1) * VS]).astype(d_fc),
            "fc_b": np.ascontiguousarray(
                fc_b_np[c * VS:(c + 1) * VS][None, :]).astype(d_fc),
        })
    return in_maps


def assemble(results):
    logits = np.concatenate([results[c]["out_logits"] for c in range(N_CORES)],
                            axis=1).astype(np.float32)
    state = np.asarray(results[0]["out_state"], np.float32)
    attn = np.concatenate([results[c]["out_attn"] for c in range(N_CORES)],
                          axis=0).astype(np.float32)[..., None]
    return logits, state, attn


_NC_CACHE = {}


def kernel(**inputs):
    key = tuple(sorted(CFG.items()))
    if key not in _NC_CACHE:
        _NC_CACHE[key] = build_nc()
    nc = _NC_CACHE[key]
    in_maps = shard_inputs(**inputs)
    res = run_bass_kernel_spmd(nc, in_maps, list(range(N_CORES)))
    return assemble(res.results)


# revision 26
# speedup vs baseline: 1.1151x; 1.1151x over previous
"""Trainium2 Bass kernel for nn_Decoder — v3 (see kernel.py docstring)."""
from contextlib import ExitStack

import numpy as np
import ml_dtypes

import concourse.bass as bass
import concourse.mybir as mybir
import concourse.tile as tile
from concourse import bacc
from concourse.bass_utils import run_bass_kernel_spmd
from concourse.masks import make_identity

F32 = mybir.dt.float32
F32R = mybir.dt.float32r
BF16 = mybir.dt.bfloat16
I32 = mybir.dt.int32
AF = mybir.ActivationFunctionType
ALU = mybir.AluOpType
AX = mybir.AxisListType

N_CORES = 8
B, T, U, E, VOCAB = 64, 128, 1024, 256, 32000
BL = B // N_CORES
R = BL * T
G2 = 2 * U
VS = VOCAB // N_CORES
NK = U // 128
NM = U // 128
NVN = 8
VN = VS // NVN
HB = 4

CFG = {
    "mm1": "bf16",
    "scores": "bf16",
    "gru": "bf16",
    "fc": "bf16",
}


def _dt(stage):
    return BF16 if CFG[stage] == "bf16" else F32R


def _np_dt(stage):
    return ml_dtypes.bfloat16 if CFG[stage] == "bf16" else np.float32


def build_nc():
    nc = bacc.Bacc("TRN2", target_bir_lowering=False, debug=False,
                   num_devices=N_CORES)

    d_mm1, d_sc, d_gru, d_fc = (_dt(s) for s in ("mm1", "scores", "gru", "fc"))

    enc_t = nc.dram_tensor("enc_t", [U, R], d_mm1, kind="ExternalInput").ap()
    w1 = nc.dram_tensor("w1", [U, U], d_mm1, kind="ExternalInput").ap()
    w2 = nc.dram_tensor("w2", [U, U], d_mm1, kind="ExternalInput").ap()
    hidden_t = nc.dram_tensor("hidden_t", [U, BL], d_mm1, kind="ExternalInput").ap()
    b1n = nc.dram_tensor("b1n", [1, U], F32, kind="ExternalInput").ap()
    b2n = nc.dram_tensor("b2n", [1, U], F32, kind="ExternalInput").ap()
    sel = nc.dram_tensor("sel", [2, BL + 1, 512], F32R, kind="ExternalInput").ap()
    v_t = nc.dram_tensor("v_t", [128, NM], d_sc, kind="ExternalInput").ap()
    emb = nc.dram_tensor("emb", [VOCAB, E], F32, kind="ExternalInput").ap()
    x_idx = nc.dram_tensor("x_idx", [B, 1], I32, kind="ExternalInput").ap()
    gru_k = nc.dram_tensor("gru_k_zh", [NK + 2, 128, G2], d_gru,
                           kind="ExternalInput").ap()
    gru_b = nc.dram_tensor("gru_b_zh", [1, G2], F32, kind="ExternalInput").ap()
    fc_w = nc.dram_tensor("fc_w", [U, VS], d_fc, kind="ExternalInput").ap()
    fc_b = nc.dram_tensor("fc_b", [1, VS], d_fc, kind="ExternalInput").ap()

    out_logits = nc.dram_tensor("out_logits", [B, VS], F32, kind="ExternalOutput").ap()
    out_state = nc.dram_tensor("out_state", [B, U], F32, kind="ExternalOutput").ap()
    out_attn = nc.dram_tensor("out_attn", [BL, T], F32, kind="ExternalOutput").ap()

    def bcast(src_ap, parts):
        inner = [list(d) for d in src_ap.ap if d[1] != 1]
        return bass.AP(tensor=src_ap.tensor, offset=src_ap.offset,
                       ap=[[0, parts]] + inner)

    with tile.TileContext(nc) as tc, ExitStack() as es:
        consts = es.enter_context(tc.tile_pool(name="consts", bufs=1))
        enc_p = es.enter_context(tc.tile_pool(name="enc", bufs=1))
        w1_p = es.enter_context(tc.tile_pool(name="w1", bufs=1))
        w2_p = es.enter_context(tc.tile_pool(name="w2", bufs=1))
        tanh_p = es.enter_context(tc.tile_pool(name="tanh", bufs=1))
        fcw_p = es.enter_context(tc.tile_pool(name="fcw", bufs=4))
        gruk_p = es.enter_context(tc.tile_pool(name="gruk", bufs=5))
        small = es.enter_context(tc.tile_pool(name="small", bufs=1))
        psum = es.enter_context(tc.tile_pool(name="psum", bufs=8, space="PSUM"))
        dram = es.enter_context(tc.tile_pool(name="dram", bufs=4, space="DRAM"))

        # ---- phase 0: loads (few, large), h2, embedding ----
        hid_sb = consts.tile([128, NK, BL], d_mm1)
        nc.sync.dma_start(out=hid_sb[:],
                          in_=hidden_t.rearrange("(k p) b -> p k b", p=128))
        w2_sb = w2_p.tile([128, NK, U], d_mm1, tag="w2", name="w2_sb")
        nc.sync.dma_start(out=w2_sb[:],
                          in_=w2.rearrange("(k p) n -> p k n", p=128))
        enc_sb = enc_p.tile([128, NK, R], d_mm1, tag="enc")
        for j in range(2):
            nc.sync.dma_start(
                out=enc_sb[:, j * 4:(j + 1) * 4, :],
                in_=enc_t[j * 512:(j + 1) * 512, :].rearrange(
                    "(k p) r -> p k r", p=128))
        w1_sb = consts.tile([128, NK, U], d_mm1)
        for j in range(2):
            nc.sync.dma_start(
                out=w1_sb[:, j * 4:(j + 1) * 4, :],
                in_=w1[j * 512:(j + 1) * 512, :].rearrange(
                    "(k p) n -> p k n", p=128))

        ident = consts.tile([128, 128], F32)
        make_identity(nc, ident[:])
        ones = consts.tile([1, B], d_fc)
        nc.vector.memset(ones[:], 1.0)
        v_sb = consts.tile([128, NM], d_sc)
        nc.sync.dma_start(out=v_sb[:], in_=v_t[:])
        sel_sb = consts.tile([BL + 1, 2, 512], F32R)
        nc.sync.dma_start(out=sel_sb[:],
                          in_=sel.rearrange("h j n -> j h n"))
        b1_sb = consts.tile([1, U], F32)
        nc.sync.dma_start(out=b1_sb[:], in_=b1n[:])
        b2_sb = consts.tile([1, U], F32)
        nc.sync.dma_start(out=b2_sb[:], in_=b2n[:])

        # h2 = hidden @ W2 (natural [BL, U]); pack into f32r-rounded h2aug:
        # rows 0-3 = h2[b0..3], row 4 = b1+b2, rows 5-8 = h2[b4..7], row 9 = b1+b2
        h2aug = consts.tile([BL + 1, U], F32R)
        ph = [psum.tile([BL, 512], F32, tag="pb", name=f"ph{n}") for n in range(2)]
        for k in range(NK):
            for n in range(2):
                nc.tensor.matmul(
                    ph[n][:],
                    hid_sb[:, k, :],
                    w2_sb[:, k, n * 512:(n + 1) * 512],
                    start=(k == 0), stop=(k == NK - 1))
        for n in range(2):
            nc.vector.tensor_copy(out=h2aug[0:BL, n * 512:(n + 1) * 512],
                                  in_=ph[n][:])
        b12 = small.tile([1, U], F32, tag="b12")
        nc.vector.tensor_add(out=b12[:], in0=b1_sb[:], in1=b2_sb[:])
        nc.sync.dma_start(out=h2aug[BL:BL + 1, :], in_=b12[:].bitcast(F32R))

        # full-batch embedding gather -> gru input chunks 8..9 [128, B]
        idx_sb = small.tile([B, 1], I32, tag="idx")
        nc.sync.dma_start(out=idx_sb[:], in_=x_idx[:])
        xe_sb = small.tile([B, E], F32, tag="xe")
        nc.gpsimd.indirect_dma_start(
            out=xe_sb[:], out_offset=None, in_=emb[:],
            in_offset=bass.IndirectOffsetOnAxis(ap=idx_sb[:, :1], axis=0))
        gx = []
        for j in range(2):
            pt = psum.tile([128, B], F32, tag="pb", name=f"xe_ps{j}")
            nc.tensor.transpose(pt[:], xe_sb[:, j * 128:(j + 1) * 128],
                                ident[:B, :B])
            g = small.tile([128, B], d_gru, tag=f"gx{j}", name=f"gx{j}")
            nc.vector.tensor_copy(out=g[:], in_=pt[:])
            gx.append(g)

        # ---- phase 1: attention, two 512-row halves ----
        tanh_sb = tanh_p.tile([128, NM, R], d_sc, tag="tanh")
        sc_dram = dram.tile([1, R], F32)
        attn_dram = dram.tile([BL, T], F32)
        st_gin = small.tile([128, NK, BL], F32, tag="stgin")
        for h in range(2):
            rows = slice(h * 512, (h + 1) * 512)
            for m in range(NM):
                ps = psum.tile([128, 512], F32, tag="pb", name=f"mm1_{h}_{m}")
                for k in range(NK):
                    nc.tensor.matmul(
                        ps[:],
                        w1_sb[:, k, m * 128:(m + 1) * 128],
                        enc_sb[:, k, rows],
                        start=(k == 0), stop=False)
                nc.tensor.matmul(
                    ps[:],
                    h2aug[:, m * 128:(m + 1) * 128],
                    sel_sb[:, h, :],
                    start=False, stop=True)
                nc.scalar.activation(out=tanh_sb[:, m, rows], in_=ps[:],
                                     func=AF.Tanh)
            psc = psum.tile([1, 512], F32, tag="pb", name=f"sc_ps{h}")
            for m in range(NM):
                nc.tensor.matmul(
                    psc[:], v_sb[:, m:m + 1], tanh_sb[:, m, rows],
                    start=(m == 0), stop=(m == NM - 1))
            sc_sb = small.tile([1, 512], F32, tag="scsb", bufs=2,
                               name=f"sc_sb{h}")
            nc.vector.tensor_copy(out=sc_sb[:], in_=psc[:])
            nc.sync.dma_start(out=sc_dram[:, rows], in_=sc_sb[:])

            sm_sb = small.tile([HB, T], F32, tag="smx", bufs=2, name=f"sm{h}")
            nc.sync.dma_start(
                out=sm_sb[:],
                in_=sc_dram[0:1, rows].rearrange("o (b t) -> (o b) t", t=T))
            # softmax without max-subtraction (scores are O(1); exp is safe)
            ex_sb = small.tile([HB, T], F32, tag="ex", bufs=2, name=f"ex{h}")
            esum = small.tile([HB, 1], F32, tag="esum", bufs=2, name=f"esum{h}")
            nc.scalar.activation(out=ex_sb[:], in_=sm_sb[:], func=AF.Exp,
                                 accum_out=esum[:, :1])
            rsum = small.tile([HB, 1], F32, tag="rsum", bufs=2, name=f"rsum{h}")
            nc.vector.reciprocal(out=rsum[:], in_=esum[:])
            attn_sb = small.tile([HB, T], F32, tag="attn", bufs=2,
                                 name=f"attn{h}")
            nc.vector.tensor_scalar_mul(out=attn_sb[:], in0=ex_sb[:],
                                        scalar1=rsum[:, :1])
            nc.sync.dma_start(out=out_attn[h * HB:(h + 1) * HB, :], in_=attn_sb[:])
            nc.sync.dma_start(out=attn_dram[h * HB:(h + 1) * HB, :], in_=attn_sb[:])

            abc = small.tile([128, HB, T], F32, tag="abc", bufs=2,
                             name=f"abc{h}")
            nc.gpsimd.dma_start(out=abc[:],
                                in_=bcast(attn_dram[h * HB:(h + 1) * HB, :], 128))
            scratch = small.tile([128, T], F32, tag="scr", bufs=2,
                                 name=f"scr{h}")
            for b in range(HB):
                gb = h * HB + b
                scr = scratch
                for k in range(NK):
                    nc.vector.scalar_tensor_tensor(
                        out=scr[:],
                        in0=enc_sb[:, k, gb * T:(gb + 1) * T],
                        scalar=1.0, in1=abc[:, b, :],
                        op0=ALU.mult, op1=ALU.mult,
                        accum_out=st_gin[:, k, gb:gb + 1])

        gb_bc = small.tile([B, G2], BF16, tag="gbbc")
        nc.gpsimd.dma_start(out=gb_bc[:], in_=bcast(gru_b[0:1, :], B))

        # ---- phase 2: all-gather of contextT ----
        st_bf = small.tile([128, NK, BL], d_gru, tag="stbf")
        nc.vector.tensor_copy(out=st_bf[:], in_=st_gin[:])
        cc_in = dram.tile([U, BL], d_gru)
        nc.sync.dma_start(out=cc_in[:].rearrange("(k p) b -> p k b", p=128),
                          in_=st_bf[:])
        cc_out = dram.tile([N_CORES, U, BL], d_gru)
        nc.gpsimd.collective_compute(
            "AllGather", ALU.bypass,
            replica_groups=[list(range(N_CORES))],
            ins=[cc_in.opt()], outs=[cc_out.opt()])

        # stream gru_k / fc_w while the collective runs (issued after the
        # collective so their transfers fill its window; sync queue — the
        # scalar HWDGE path is broken on this stack)
        gruk_sb = []
        for k in range(NK + 2):
            gt = gruk_p.tile([128, G2], d_gru, tag="gruk", name=f"gruk{k}")
            nc.sync.dma_start(out=gt[:], in_=gru_k[k])
            gruk_sb.append(gt)
        fcw_sb = []
        for k in range(NK):
            t = fcw_p.tile([128, VS], d_fc, tag="fcw", name=f"fcw{k}")
            nc.sync.dma_start(out=t[:], in_=fc_w[k * 128:(k + 1) * 128, :])
            fcw_sb.append(t)
        fcb_sb = small.tile([1, VS], d_fc, tag="fcb")
        nc.sync.dma_start(out=fcb_sb[:], in_=fc_b[:])
        gf_all = small.tile([128, NK, B], d_gru, tag="gfall")
        for k in range(NK):
            nc.sync.dma_start(
                out=gf_all[:, k, :].rearrange("p (r b) -> p r b", r=N_CORES),
                in_=cc_out[:, k * 128:(k + 1) * 128, :].rearrange(
                    "r p b -> p r b"))

        # ---- phase 3: full-batch GRU (replicated on every core) ----
        gin_ap = [gf_all[:, k, :] for k in range(NK)] + [gx[0][:], gx[1][:]]
        pg = [psum.tile([B, 512], F32, tag="pb", name=f"pg{n}")
              for n in range(4)]
        for k in range(NK + 2):
            for n in range(4):
                nc.tensor.matmul(
                    pg[n][:], gin_ap[k],
                    gruk_sb[k][:, n * 512:(n + 1) * 512],
                    start=(k == 0), stop=(k == NK + 1))
        z_sb = small.tile([B, U], F32, tag="z")
        hh_sb = small.tile([B, U], F32, tag="hh")
        for n in range(4):
            gsum = small.tile([B, 512], F32, tag="gsum", bufs=2,
                              name=f"gsum{n}")
            nc.vector.tensor_add(out=gsum[:], in0=pg[n][:],
                                 in1=gb_bc[:, n * 512:(n + 1) * 512])
            if n < 2:
                nc.scalar.activation(out=z_sb[:, n * 512:(n + 1) * 512],
                                     in_=gsum[:], func=AF.Sigmoid)
            else:
                nc.scalar.activation(out=hh_sb[:, (n - 2) * 512:(n - 1) * 512],
                                     in_=gsum[:], func=AF.Tanh)
        state_sb = small.tile([B, U], F32, tag="state")
        nc.vector.tensor_mul(out=z_sb[:], in0=z_sb[:], in1=hh_sb[:])
        nc.vector.tensor_sub(out=state_sb[:], in0=hh_sb[:], in1=z_sb[:])
        nc.sync.dma_start(out=out_state[:], in_=state_sb[:])

        stT = []
        for m in range(NM):
            pt = psum.tile([128, B], F32, tag="pb", name=f"st_ps{m}")
            nc.tensor.transpose(pt[:], state_sb[:, m * 128:(m + 1) * 128],
                                ident[:B, :B])
            sb_ = small.tile([128, B], d_fc, tag="sTb", bufs=NM,
                             name=f"stT{m}")
            nc.vector.tensor_copy(out=sb_[:], in_=pt[:])
            stT.append(sb_)

        # ---- phase 4: fc projection ----
        lo_all = small.tile([B, VS], F32, tag="loall")
        pl = [psum.tile([B, VN], F32, tag="pb", name=f"pl{n}")
              for n in range(NVN)]
        for k in range(NK):
            for n in range(NVN):
                nc.tensor.matmul(
                    pl[n][:], stT[k][:],
                    fcw_sb[k][:, n * VN:(n + 1) * VN],
                    start=(k == 0), stop=False)
        for n in range(NVN):
            nc.tensor.matmul(
                pl[n][:], ones[:], fcb_sb[:, n * VN:(n + 1) * VN],
                start=False, stop=True)
            nc.vector.tensor_copy(out=lo_all[:, n * VN:(n + 1) * VN],
                                  in_=pl[n][:])
        nc.sync.dma_start(out=out_logits[:], in_=lo_all[:])

    nc.compile()
    return nc


def shard_inputs(x, hidden, enc_output, emb, W1, b1, W2, b2, V, bV,
                 gru_k, gru_rk, gru_b, fc_W, fc_b):
    f32 = np.float32
    d_mm1, d_sc, d_gru, d_fc = (_np_dt(s) for s in ("mm1", "scores", "gru", "fc"))

    x = np.asarray(x).astype(np.int32).reshape(B, 1)
    hidden = np.asarray(hidden, f32)
    enc_output = np.asarray(enc_output, f32)
    emb_np = np.ascontiguousarray(np.asarray(emb, f32))
    W1_np = np.ascontiguousarray(np.asarray(W1, f32).astype(d_mm1))
    W2_np = np.ascontiguousarray(np.asarray(W2, f32).astype(d_mm1))
    b1n = np.ascontiguousarray(np.asarray(b1, f32)[None, :])
    b2n = np.ascontiguousarray(np.asarray(b2, f32)[None, :])
    selv = np.zeros((2, BL + 1, 512), f32)
    for h in range(2):
        for j in range(HB):
            selv[h, h * HB + j, j * T:(j + 1) * T] = 1.0
        selv[h, BL, :] = 1.0
    v_t = np.ascontiguousarray(np.asarray(V, f32)[:, 0].reshape(NM, 128).T
                               .astype(d_sc))
    gk = np.asarray(gru_k, f32)
    gru_k_zh = np.ascontiguousarray(
        np.concatenate([gk[:, :U], gk[:, 2 * U:]], axis=1).astype(d_gru)
        .reshape(NK + 2, 128, G2))
    gb = np.asarray(gru_b, f32)
    gru_b_zh = np.ascontiguousarray(
        np.concatenate([gb[:U], gb[2 * U:]])[None, :].astype(f32))
    fc_W_np = np.asarray(fc_W, f32)
    fc_b_np = np.asarray(fc_b, f32)

    in_maps = []
    for c in range(N_CORES):
        sl = slice(c * BL, (c + 1) * BL)
        enc_c = enc_output[sl].reshape(R, U)
        in_maps.append({
            "enc_t": np.ascontiguousarray(enc_c.T).astype(d_mm1),
            "w1": W1_np,
            "w2": W2_np,
            "hidden_t": np.ascontiguousarray(hidden[sl].T).astype(d_mm1),
            "b1n": b1n,
            "b2n": b2n,
            "sel": selv,
            "v_t": v_t,
            "emb": emb_np,
            "x_idx": x,
            "gru_k_zh": gru_k_zh,
            "gru_b_zh": gru_b_zh,
            "fc_w": np.ascontiguousarray(
                fc_W_np[:, c * VS:(c + 1) * VS]).astype(d_fc),
            "fc_b": np.ascontiguousarray(
                fc_b_np[c * VS:(c + 1) * VS][None, :]).astype(d_fc),
        })
    return in_maps


def assemble(results):
    logits = np.concatenate([results[c]["out_logits"] for c in range(N_CORES)],
                            axis=1).astype(np.float32)
    state = np.asarray(results[0]["out_state"], np.float32)
    attn = np.concatenate([results[c]["out_attn"] for c in range(N_CORES)],
                          axis=0).astype(np.float32)[..., None]
    return logits, state, attn


_NC_CACHE = {}


def kernel(**inputs):
    key = tuple(sorted(CFG.items()))
    if key not in _NC_CACHE:
        _NC_CACHE[key] = build_nc()
    nc = _NC_CACHE[key]
    in_maps = shard_inputs(**inputs)
    res = run_bass_kernel_spmd(nc, in_maps, list(range(N_CORES)))
    return assemble(res.results)
